# revision 1
# baseline (speedup 1.0000x reference)
"""Differentiable risk budgeting solve on 8 Trainium2 NeuronCores.

Problem: 20 unrolled iterations of
    Sw   = einsum('bij,bj->bi', sigma, w)
    grad = 2*Sw - beta + lam_s*sign(w) + 2*lam_t*(w - w_prev)
    w    = proj(w - 0.05*grad)          # clip/renorm twice
with B=32768, P=45.

Strategy: pure data parallel over 8 cores (4096 batch rows each).  Per
core, batch tiles of 512 rows (128 partitions x 4 groups) keep sigma
SBUF-resident for all 20 iterations, so HBM traffic is one pass over
sigma.  The batched matvec runs on the VectorEngine as an elementwise
multiply (w broadcast over rows of sigma) followed by a segmented
reduction over the contraction axis; per-batch matrices cannot share
TensorEngine weights, so the PE is not usable here.

The update is algebraically folded to
    w' = cw*w - 0.1*Sw - s*sign(w) + D
with cw = 1-0.1*lam_t, s = 0.05*lam_s, D = 0.05*beta + 0.1*lam_t*w_prev
(D computed on host, lam's baked as immediates at trace time), and
sign(w>=0) realized branch-free as min(w*1e30, s).
"""

import os
import sys

sys.path.insert(0, "/opt/trn_rl_repo")

import numpy as np

import concourse.bacc as bacc
import concourse.bass as bass
import concourse.mybir as mybir
import concourse.tile as tile
from concourse.bass_utils import run_bass_kernel_spmd

N_CORES = 8
B_TOTAL = 32768
P = 45
BC = B_TOTAL // N_CORES  # 4096 batch rows per core

N_ITER = 20
STEP = 0.05
MAXW = 0.15
EPS = 1e-8
BIG = 1.0e30

NB = 4  # batch groups per tile (free dim)
TB = 128 * NB  # batch rows per tile
NT = BC // TB  # tiles per core

F32 = mybir.dt.float32
ALU = mybir.AluOpType


def _build_program(cw: float, s: float):
    """Trace the per-core Bass program. cw/s are baked as immediates."""
    nc = bacc.Bacc("TRN2", target_bir_lowering=False, debug=False)

    sig_d = nc.dram_tensor("sigma", [BC, P, P], F32, kind="ExternalInput").ap()
    d_d = nc.dram_tensor("dvec", [BC, P], F32, kind="ExternalInput").ap()
    w_d = nc.dram_tensor("wout", [BC, P], F32, kind="ExternalOutput").ap()

    c0 = float(np.float32(cw) / np.float32(P) - np.float32(s))
    reps = int(os.environ.get("RISK_KERNEL_BENCH_REPS", "1"))

    import contextlib

    with tile.TileContext(nc) as tc:
        with (
            tc.tile_pool(name="sig", bufs=2) as psig,
            tc.tile_pool(name="prod", bufs=1) as pprod,
            tc.tile_pool(name="wrk", bufs=2) as pwrk,
            tc.For_i(0, reps, 1) if reps > 1 else contextlib.nullcontext(),
        ):
            for t in range(NT):
                base = t * TB
                # ---- load sigma tile: [128, NB, P, P], one DMA per group ----
                sig = psig.tile([128, NB * P * P], F32, tag="sig")
                sig4 = sig[:].rearrange("p (g i j) -> p g i j", g=NB, i=P)
                for g in range(NB):
                    nc.gpsimd.dma_start(
                        sig4[:, g], sig_d[base + g * 128 : base + (g + 1) * 128]
                    )
                # load D tile
                dt_ = pwrk.tile([128, NB * P], F32, tag="d")
                dt3 = dt_[:].rearrange("p (g j) -> p g j", g=NB)
                for g in range(NB):
                    nc.gpsimd.dma_start(
                        dt3[:, g], d_d[base + g * 128 : base + (g + 1) * 128]
                    )

                # ---- pre-scale sigma by -0.1 (ScalarE, off critical path) ----
                nc.scalar.mul(sig[:], sig[:], -STEP * 2.0)

                prod = pprod.tile([128, NB * P * P], F32, tag="prod")
                prod4 = prod[:].rearrange("p (g i j) -> p g i j", g=NB, i=P)

                w = pwrk.tile([128, NB * P], F32, tag="w")
                w3 = w[:].rearrange("p (g j) -> p g j", g=NB)
                swp = pwrk.tile([128, NB * P], F32, tag="swp")
                swp3 = swp[:].rearrange("p (g i) -> p g i", g=NB)
                sgn = pwrk.tile([128, NB * P], F32, tag="sgn")
                e1 = pwrk.tile([128, NB * P], F32, tag="e1")
                u = pwrk.tile([128, NB * P], F32, tag="u")
                wc = pwrk.tile([128, NB * P], F32, tag="wc")
                wc3 = wc[:].rearrange("p (g j) -> p g j", g=NB)
                r = pwrk.tile([128, NB], F32, tag="r")
                rr = pwrk.tile([128, NB], F32, tag="rr")

                u3 = u[:].rearrange("p (g j) -> p g j", g=NB)

                def project(src_ap, dst3_ap):
                    # dst = proj(src): clip -> renorm -> clip -> renorm
                    rrb = rr[:].unsqueeze(2).broadcast_to([128, NB, P])
                    nc.vector.tensor_scalar(wc[:], src_ap, 0.0, MAXW, ALU.max, ALU.min)
                    nc.vector.tensor_reduce(r[:], wc3, mybir.AxisListType.X, ALU.add)
                    nc.vector.tensor_scalar(r[:], r[:], EPS, None, ALU.add)
                    nc.vector.reciprocal(rr[:], r[:])
                    nc.vector.tensor_tensor(u3, wc3, rrb, ALU.mult)
                    nc.vector.tensor_scalar(wc[:], u[:], 0.0, MAXW, ALU.max, ALU.min)
                    nc.vector.tensor_reduce(r[:], wc3, mybir.AxisListType.X, ALU.add)
                    nc.vector.tensor_scalar(r[:], r[:], EPS, None, ALU.add)
                    nc.vector.reciprocal(rr[:], r[:])
                    nc.vector.tensor_tensor(dst3_ap, wc3, rrb, ALU.mult)

                # ---- iteration 0 (w0 = 1/P): u0 = reduce(sig')/P + c0 + D ----
                nc.vector.tensor_reduce(swp3, sig4, mybir.AxisListType.X, ALU.add)
                nc.vector.tensor_scalar(
                    u[:], swp[:], 1.0 / P, c0, ALU.mult, ALU.add
                )
                nc.vector.tensor_tensor(u[:], u[:], dt_[:], ALU.add)
                project(u[:], w3)

                # ---- iterations 1..N-1 ----
                for it in range(1, N_ITER):
                    wb = (
                        w3.unsqueeze(2).broadcast_to([128, NB, P, P])
                    )  # w[p,g,j] -> [p,g,i,j]
                    nc.vector.tensor_tensor(prod4, sig4, wb, ALU.mult)
                    nc.vector.tensor_reduce(
                        swp3, prod4, mybir.AxisListType.X, ALU.add
                    )
                    nc.vector.tensor_scalar(sgn[:], w[:], BIG, s, ALU.mult, ALU.min)
                    nc.vector.tensor_tensor(e1[:], dt_[:], sgn[:], ALU.subtract)
                    # e1 += cw*w, fused: (w mult cw) add e1
                    nc.vector.scalar_tensor_tensor(
                        e1[:], w[:], cw, e1[:], ALU.mult, ALU.add
                    )
                    nc.vector.tensor_tensor(u[:], e1[:], swp[:], ALU.add)
                    project(u[:], w3)

                # ---- store ----
                for g in range(NB):
                    nc.gpsimd.dma_start(
                        w_d[base + g * 128 : base + (g + 1) * 128], w3[:, g]
                    )

    nc.compile()
    return nc


def kernel(sigma, beta, w_prev, log_lambda_sparse, log_lambda_turnover):
    sigma = np.ascontiguousarray(sigma, dtype=np.float32)
    beta = np.asarray(beta, dtype=np.float32)
    w_prev = np.asarray(w_prev, dtype=np.float32)

    lam_s = np.exp(np.float32(log_lambda_sparse), dtype=np.float32)
    lam_t = np.exp(np.float32(log_lambda_turnover), dtype=np.float32)
    cw = float(np.float32(1.0) - np.float32(2 * STEP) * lam_t)
    s = float(np.float32(STEP) * lam_s)
    dvec = (
        np.float32(STEP) * beta + np.float32(2 * STEP) * lam_t * w_prev
    ).astype(np.float32)

    nc = _build_program(cw, s)

    in_maps = []
    for c in range(N_CORES):
        sl = slice(c * BC, (c + 1) * BC)
        in_maps.append({"sigma": sigma[sl], "dvec": dvec[sl]})

    res = run_bass_kernel_spmd(nc, in_maps, core_ids=list(range(N_CORES)))
    out = np.concatenate([res.results[c]["wout"] for c in range(N_CORES)], axis=0)
    return out.astype(np.float32)


if __name__ == "__main__":
    rng = np.random.default_rng(0)
    A = rng.standard_normal((B_TOTAL, P, P), dtype=np.float32) * 0.1
    sig = np.einsum("bij,bkj->bik", A, A) + 0.1 * np.eye(P, dtype=np.float32)
    bet = rng.random((B_TOTAL, P), dtype=np.float32)
    bet /= bet.sum(-1, keepdims=True)
    wp = np.full((B_TOTAL, P), 1.0 / P, dtype=np.float32)
    out = kernel(
        sigma=sig,
        beta=bet,
        w_prev=wp,
        log_lambda_sparse=np.float32(-3.0),
        log_lambda_turnover=np.float32(-2.0),
    )
    print(out.shape, out.dtype, out[:2, :5])

